# revision 34
# baseline (speedup 1.0000x reference)
"""Multi-head attention Trainium2 kernel (B=2, L=2048, H=16, dk=dv=64).

Sharding: 8 cores; core c handles batch c//4, heads 4*(c%4) .. 4*(c%4)+3.

Per-core algorithm (transposed-scores layout — no per-head attn transposes):
  - Q/K transposed on-chip via PE per head-pair (partitions 0-63 = even head
    dims, 64-127 = odd head dims), Q pre-scaled by 1/sqrt(dk), stored as
    bf16 hi/lo pairs (bf16x2 split: near-f32 scores from 3 bf16 matmuls).
  - mask[b] cast u8->bf16 into DRAM scratch (SWDGE), then transposed per
    128-key tile via xbar DMA-transpose and inverted on-chip (1-m).
  - scoresT[k, q] per (head-pair, 512-q chunk, key-tile): 2-head row-packed
    matmuls (contraction 64, tile rows 0-63 / 64-127); softmax without
    max-subtraction (safe at these magnitudes): exp on ACT (psum -> sbuf
    bf16), multiplicative mask on DVE (bf16 2x), attn @ V accumulated in
    psum with a ones-column on V providing the softmax denominators.
  - transpose-back via PE, normalize (reciprocal * scale) on DVE, store.
"""

import os
import threading

import numpy as np

import concourse.bass as bass
import concourse.tile as tile
from concourse import bacc, mybir
from concourse.masks import make_identity

F32 = mybir.dt.float32
BF16 = mybir.dt.bfloat16
U16 = mybir.dt.uint16
U8 = mybir.dt.uint8
AF = mybir.ActivationFunctionType
ALU = mybir.AluOpType

NUM_HEADS = 16
DK = 64
B = 2
L_FULL = 2048
N_CORES = 8
HC = 4           # heads per core
QK_MODE = os.environ.get("QK_MODE", "bf16x2")   # "bf16" | "bf16x2"


def build_attention_tile(nc, tc, q_in, k_in, v_in, m_in, o_out, L, HC):
    """Trace the per-core attention program into TileContext tc.

    q_in/k_in/v_in/o_out: [L, HC*64] f32 DRAM APs. m_in: [L, L] u8 DRAM AP —
    the TRANSPOSED mask for this batch (m_in[k, q] = mask[b, q, k]).
    """
    from contextlib import ExitStack

    HP = HC // 2          # head pairs
    NT = L // 128         # key tiles (128 keys each)
    QB = L // 512         # query chunks (512 q each)
    NCH = L // 128        # 128-row chunks
    split = QK_MODE == "bf16x2"

    with ExitStack() as ctx:
        singles = ctx.enter_context(tc.tile_pool(name="singles", bufs=1))
        ident = singles.tile([128, 128], F32)
        make_identity(nc, ident)
        ident_bf = singles.tile([128, 128], BF16)
        make_identity(nc, ident_bf)

        qkt = ctx.enter_context(tc.tile_pool(name="qkt", bufs=1))
        qt_hi = [qkt.tile([128, L], BF16, tag=f"qh{h}", name=f"qh{h}")
                 for h in range(HP)]
        kt_hi = [qkt.tile([128, L], BF16, tag=f"kh{h}", name=f"kh{h}")
                 for h in range(HP)]
        if split:
            qt_lo = [qkt.tile([128, L], BF16, tag=f"ql{h}", name=f"ql{h}")
                     for h in range(HP)]
            kt_lo = [qkt.tile([128, L], BF16, tag=f"kl{h}", name=f"kl{h}")
                     for h in range(HP)]

        mi_pool = ctx.enter_context(tc.tile_pool(name="mi", bufs=1))
        mi = [mi_pool.tile([128, L], BF16, tag=f"mi{j}", name=f"mi{j}")
              for j in range(NT)]

        vp_pool = ctx.enter_context(tc.tile_pool(name="vp", bufs=1))
        vp = [vp_pool.tile([128, HC * 65], BF16, tag=f"vp{j}", name=f"vp{j}")
              for j in range(NT)]

        # ---------------- prep phase ----------------
        with tc.tile_pool(name="prep_ps", bufs=2, space="PSUM") as prep_ps, \
             tc.tile_pool(name="prep_sb", bufs=1) as prep_sb:
            # Q/K staging loads first: they have no deps, so the sync
            # queue starts them immediately (the mask xbars below gate
            # on the SWDGE cast and would head-of-line-block them).
            stg_dt = F32 if split else BF16
            stgs = {}
            for hp in range(HP):
                for nm, src in (("q", q_in), ("k", k_in)):
                    stg = prep_sb.tile([128, NCH, 128], stg_dt,
                                       tag=f"stg{nm}{hp}",
                                       name=f"stg{nm}{hp}")
                    src_ap = src[:, 128 * hp:128 * hp + 128].rearrange(
                        "(c p) w -> p c w", p=128)
                    # split into 4 pieces so the PE transposes (which
                    # consume per-chunk) can start on the first piece
                    for pc in range(4):
                        c0 = NCH // 4 * pc
                        c1 = NCH // 4 * (pc + 1)
                        if split:
                            nc.sync.dma_start(out=stg[:, c0:c1, :],
                                              in_=src_ap[:, c0:c1, :])
                        else:
                            # SWDGE cast f32 -> bf16 during the load
                            nc.gpsimd.dma_start(out=stg[:, c0:c1, :],
                                                in_=src_ap[:, c0:c1, :])
                    stgs[(nm, hp)] = stg
            # Q/K transposes: per head pair, [L, 128] -> [128, L]
            for hp in range(HP):
                for nm, hi_dst, lo_dst, scale in (
                        ("q", qt_hi[hp], qt_lo[hp] if split else None, 0.125),
                        ("k", kt_hi[hp], kt_lo[hp] if split else None, None)):
                    stg = stgs[(nm, hp)]
                    pst = prep_ps.tile([128, L], stg_dt, tag="pst",
                                       name="pst")
                    for c in range(NCH):
                        nc.tensor.transpose(
                            pst[:, 128 * c:128 * (c + 1)], stg[:, c, :],
                            ident if split else ident_bf)
                    if scale is not None:
                        nc.vector.tensor_scalar_mul(hi_dst, pst, scale)
                    else:
                        nc.vector.tensor_copy(hi_dst, pst)
                    if split:
                        # lo = x - hi (x optionally pre-scaled)
                        if scale is not None:
                            sc = prep_sb.tile([128, L], F32, tag="sc",
                                              name="sc")
                            nc.vector.tensor_scalar_mul(sc, pst, scale)
                            nc.vector.tensor_tensor(lo_dst, sc, hi_dst,
                                                    ALU.subtract)
                        else:
                            nc.vector.tensor_tensor(lo_dst, pst, hi_dst,
                                                    ALU.subtract)

            # mask pipeline: cast-load transposed mask rows (u8 -> bf16 via
            # SWDGE) + on-chip invert, then V loads for the same range.
            for j in range(NT):
                mt = mi[j]
                nc.gpsimd.dma_start(out=mt, in_=m_in[128 * j:128 * (j + 1), :])
                # invert: 1 - m  (bf16 single-src, 4x mode)
                nc.vector.tensor_scalar(mt, mt, -1.0, 1.0, ALU.mult, ALU.add)
                # V: 4 heads + ones column, f32 -> bf16 cast during SWDGE
                vt = vp[j]
                vt3 = vt.rearrange("p (h w) -> p h w", w=65)
                in_ap = v_in[128 * j:128 * (j + 1), :].rearrange(
                    "p (h w) -> p h w", w=64)
                nc.gpsimd.dma_start(out=vt3[:, :, 0:64], in_=in_ap)
                nc.vector.memset(vt3[:, :, 64:65], 1.0)

        # ---------------- main loop ----------------
        sc_pool = ctx.enter_context(tc.tile_pool(name="scps", bufs=2,
                                                 space="PSUM"))
        ot_pool = ctx.enter_context(tc.tile_pool(name="otps", bufs=1,
                                                 space="PSUM"))
        otb_pool = ctx.enter_context(tc.tile_pool(name="otbps", bufs=2,
                                                  space="PSUM"))
        ae_pool = ctx.enter_context(tc.tile_pool(name="ae", bufs=3))
        au_pool = ctx.enter_context(tc.tile_pool(name="au", bufs=3))
        ots_pool = ctx.enter_context(tc.tile_pool(name="ots", bufs=2))
        rc_pool = ctx.enter_context(tc.tile_pool(name="rc", bufs=2))
        ob_pool = ctx.enter_context(tc.tile_pool(name="ob", bufs=3))

        def emit_evac(hp, qc, otss):
            # transpose-back + normalize + store for a finished (hp, qc)
            obs = [ob_pool.tile([128, 128], F32, tag=f"ob{s}",
                                name=f"ob{s}") for s in range(4)]
            for s in range(4):
                for h in (0, 1):
                    otb = otb_pool.tile([128, 65], F32, name="otb")
                    nc.tensor.transpose(
                        otb, otss[h][:, 128 * s:128 * (s + 1)],
                        ident[0:65, 0:65])
                    rc = rc_pool.tile([128, 1], F32, name="rc")
                    nc.vector.reciprocal(rc, otb[:, 64:65])
                    nc.vector.tensor_scalar_mul(
                        obs[s][:, 64 * h:64 * h + 64], otb[:, 0:64], rc)
                nc.sync.dma_start(
                    out=o_out[512 * qc + 128 * s:512 * qc + 128 * (s + 1),
                              128 * hp:128 * hp + 128],
                    in_=obs[s])

        pending = None
        for hp in range(HP):
            for qc in range(QB):
                otps = [ot_pool.tile([65, 512], F32, tag=f"ot{h}",
                                     name=f"ot{h}") for h in (0, 1)]
                for j in range(NT):
                    # scoresT tile: [keys 128, 2 heads x 512 q] (2 banks)
                    scps = sc_pool.tile([128, 1024], F32, name="scps")
                    # emit row-packed pairs adjacently: (A_i, B_i) overlap
                    # in the PE array (row groups 0-63 / 64-127)
                    nsteps = 3 if split else 1
                    for step in range(nsteps):
                        for h in (0, 1):
                            kh = kt_hi[hp][64 * h:64 * h + 64,
                                           128 * j:128 * (j + 1)]
                            qh = qt_hi[hp][64 * h:64 * h + 64,
                                           512 * qc:512 * qc + 512]
                            if split:
                                kl = kt_lo[hp][64 * h:64 * h + 64,
                                               128 * j:128 * (j + 1)]
                                ql = qt_lo[hp][64 * h:64 * h + 64,
                                               512 * qc:512 * qc + 512]
                                lhs, rhs = ((kh, qh), (kl, qh),
                                            (kh, ql))[step]
                            else:
                                lhs, rhs = kh, qh
                            nc.tensor.matmul(
                                out=scps[:, 512 * h:512 * (h + 1)],
                                lhsT=lhs, rhs=rhs,
                                start=(step == 0),
                                stop=(step == nsteps - 1),
                                tile_position=(64 * h, 0))
                    ae = ae_pool.tile([128, 1024], BF16, name="ae")
                    nc.scalar.activation(out=ae, in_=scps, func=AF.Exp)
                    au = au_pool.tile([128, 1024], BF16, name="au")
                    mi_s = mi[j][:, 512 * qc:512 * qc + 512]
                    nc.vector.tensor_tensor(
                        au.rearrange("p (h x) -> p h x", h=2),
                        ae.rearrange("p (h x) -> p h x", h=2),
                        mi_s.unsqueeze(1).broadcast_to([128, 2, 512]),
                        ALU.mult)
                    for h in (0, 1):
                        nc.tensor.matmul(
                            out=otps[h],
                            lhsT=vp[j][:, 65 * (2 * hp + h):
                                       65 * (2 * hp + h) + 65],
                            rhs=au[:, 512 * h:512 * (h + 1)],
                            start=(j == 0), stop=(j == NT - 1))
                    # interleave the previous chunk's output stage into the
                    # middle of this j-loop so it never clumps on the PE
                    if j == 6 and pending is not None:
                        emit_evac(*pending)
                        pending = None
                # evacuate psum accumulators to SBUF; defer the rest
                otss = [ots_pool.tile([65, 512], F32, tag=f"ots{h}",
                                      name=f"ots{h}") for h in (0, 1)]
                for h in (0, 1):
                    nc.vector.tensor_copy(otss[h], otps[h])
                pending = (hp, qc, otss)
        emit_evac(*pending)


def _build_nc(L=L_FULL, HC_=HC):
    nc = bacc.Bacc("TRN2", target_bir_lowering=False, debug=False,
                   enable_asserts=False)
    q_in = nc.dram_tensor("q", [L, HC_ * DK], F32, kind="ExternalInput").ap()
    k_in = nc.dram_tensor("k", [L, HC_ * DK], F32, kind="ExternalInput").ap()
    v_in = nc.dram_tensor("v", [L, HC_ * DK], F32, kind="ExternalInput").ap()
    m_in = nc.dram_tensor("m", [L, L], U8, kind="ExternalInput").ap()
    o_out = nc.dram_tensor("o", [L, HC_ * DK], F32, kind="ExternalOutput").ap()
    with tile.TileContext(nc) as tc:
        build_attention_tile(nc, tc, q_in, k_in, v_in, m_in, o_out, L, HC_)
    nc.compile()
    return nc


_nc_cache = {}
_nc_lock = threading.Lock()


def _get_nc():
    with _nc_lock:
        if "nc" not in _nc_cache:
            _nc_cache["nc"] = _build_nc()
        return _nc_cache["nc"]


def make_in_maps(Q, K, V, mask):
    mask = np.asarray(mask)
    # transposed mask per batch (mT[k, q] = mask[b, q, k]), shared by the
    # 4 cores of each batch
    mT = [np.ascontiguousarray(mask[b].T).view(np.uint8) for b in range(B)]
    in_maps = []
    for c in range(N_CORES):
        b, g = divmod(c, N_CORES // B)
        cs = 256 * g
        in_maps.append({
            "q": np.ascontiguousarray(Q[b, :, cs:cs + 256], dtype=np.float32),
            "k": np.ascontiguousarray(K[b, :, cs:cs + 256], dtype=np.float32),
            "v": np.ascontiguousarray(V[b, :, cs:cs + 256], dtype=np.float32),
            "m": mT[b],
        })
    return in_maps


def kernel(Q, K, V, mask):
    """Full-input entry point. Q/K/V: [2, 2048, 1024] f32;
    mask: [2, 2048, 2048] bool. Returns [2, 2048, 1024] f32."""
    from concourse.bass_utils import run_bass_kernel_spmd

    nc = _get_nc()
    in_maps = make_in_maps(np.asarray(Q), np.asarray(K), np.asarray(V), mask)
    res = run_bass_kernel_spmd(nc, in_maps, core_ids=list(range(N_CORES)))
    out = np.empty((B, L_FULL, NUM_HEADS * DK), dtype=np.float32)
    for c in range(N_CORES):
        b, g = divmod(c, N_CORES // B)
        out[b, :, 256 * g:256 * g + 256] = res.results[c]["o"]
    return out
